# revision 24
# baseline (speedup 1.0000x reference)
"""CapsPrimary2d Trainium2 kernel (8-core data-parallel).

Computation (see problem reference): 9x9 stride-2 conv (B=32, CIN=256,
COUT=256, 64x64 -> 28x28), Hinton squash, 3 degenerate dynamic-routing
iterations, outputs (poses, acts).

Design notes:
 - Batch sharded 4 samples/core across 8 NeuronCores (pure data parallel).
 - Conv as tap-accumulated matmuls: for each of the 81 kernel taps and each
   128-wide cin chunk, out[co, pos] += W[tap][ci, co].T @ x[ci, window(tap)].
   Weights are host-pretransposed to [cin, cout] per tap; x windows are read
   straight from the resident x tile with strided access patterns (stride-2
   cols, row-pair rows), so no im2col copies exist on chip.
 - All matmul inputs are bf16 (cast on host), accumulation fp32 in PSUM.
 - Conv runs in 4 passes (sample-pair x cout-half) of 324 weight loads x 4
   matmuls each; a pair's routing overlaps the next pair's conv on DVE/ACT/
   GPSIMD while the PE streams uninterrupted.
 - Routing runs position-major ([pos partitions, caps-type free]) after a
   DVE 32x32 block transpose of the conv output. The routing recurrence
   collapses algebraically to per-(type, position) scalars:
     votes = g0*caps, v2 = g0^2*caps2, vb = g0*(caps.bias), b2 = |bias|^2
     s2 = c^2*v2 + 2c*vb + b2,  g = squashf(s2),  dot = g*(c*v2 + vb)
   so the 3-iteration loop touches only [pos, 32] arrays; the only full-width
   work is caps^2 / caps*bias reductions and the final poses expansion
     poses = (g*c*g0)*caps + g*bias.
 - Outputs are written in kernel-convenient layouts and rearranged on host.
"""

import os
import sys

sys.path.insert(0, "/opt/trn_rl_repo")

import numpy as np
import ml_dtypes

B, CIN, H, W = 32, 256, 64, 64
KK, STRIDE = 9, 2
C_TYPES, OS0 = 32, 8
OH = OW = 28
POS = OH * OW          # 784
POSP = 800             # padded to 32-multiple for block transposes
NSLOT = 7              # ceil(POSP / 128) position slots in pos-major tiles
NCORES = 8
SPC = B // NCORES      # samples per core = 4
NTAP = KK * KK         # 81
TAPG = 9               # taps per streamed weight group
EPS = 1e-8

_PROG = None


def _patch_act_tables():
    """Steer bacc's activation-table chooser to the one set that contains
    every function this kernel uses (Identity/Ln/Exp), so the whole program
    needs a single LoadActFuncSet instead of one 1.3us reload per Sqrt<->Exp
    style alternation. Set ids stay canonical — we only hide the functions
    from the other sets during selection."""
    import concourse.bacc as bacc_mod
    import concourse.mybir as mybir
    from concourse.hw_specs import get_activation_tables as real_gat

    AF = mybir.ActivationFunctionType
    keep = "natural_log_exp_and_others"
    drop = {AF.Exp, AF.Ln, AF.Identity, AF.Copy}

    def patched(arch):
        tabs = real_gat(arch)
        assert keep in tabs and drop - {AF.Copy} <= tabs[keep]
        return {name: (set(fs) if name == keep else set(fs) - drop)
                for name, fs in tabs.items()}

    bacc_mod.get_activation_tables = patched


def _build_program():
    import concourse.bass as bass
    import concourse.bacc as bacc
    import concourse.tile as tile
    import concourse.mybir as mybir

    _patch_act_tables()

    f32 = mybir.dt.float32
    bf16 = mybir.dt.bfloat16
    AF = mybir.ActivationFunctionType
    ALU = mybir.AluOpType
    X = mybir.AxisListType.X

    nc = bacc.Bacc("TRN2", target_bir_lowering=False, debug=False)

    x_d = nc.dram_tensor("xb", [SPC, 2, 128, H * W], bf16, kind="ExternalInput")
    w_d = nc.dram_tensor("wt", [2, NTAP, 2, 128, 128], bf16, kind="ExternalInput")
    rb_d = nc.dram_tensor("rb", [128, NSLOT, 256], f32, kind="ExternalInput")
    cb_d = nc.dram_tensor("cb", [128, 2], f32, kind="ExternalInput")
    poses_d = nc.dram_tensor("poses", [SPC, 128, NSLOT, 256], f32, kind="ExternalOutput")
    acts_d = nc.dram_tensor("acts", [SPC, 128, NSLOT, 32], f32, kind="ExternalOutput")

    def stt(out, in0, scalar, in1, op0=ALU.mult, op1=ALU.add):
        nc.vector.scalar_tensor_tensor(out=out, in0=in0, scalar=float(scalar),
                                       in1=in1, op0=op0, op1=op1)

    with tile.TileContext(nc) as tc:
        with (
            tc.tile_pool(name="consts", bufs=1) as consts,
            tc.tile_pool(name="xp", bufs=1) as xp,
            tc.tile_pool(name="wp", bufs=2) as wp,
            tc.tile_pool(name="cpm", bufs=1) as cpmp,
            tc.tile_pool(name="ccm", bufs=2) as ccmp,
            tc.tile_pool(name="big", bufs=1) as bigp,
            tc.tile_pool(name="sc", bufs=1) as scp,
            tc.tile_pool(name="ps", bufs=8, space="PSUM") as psp,
        ):
            # ---- constants / resident inputs ----
            rb_pm = consts.tile([128, NSLOT, 256], f32)
            eps_sb = consts.tile([128, 1], f32)
            nc.vector.memset(eps_sb[:], EPS)
            tiny_sb = consts.tile([128, 1], f32)
            nc.vector.memset(tiny_sb[:], 1e-30)
            one_sb = consts.tile([128, 1], f32)
            nc.vector.memset(one_sb[:], 1.0)
            cb_sb = consts.tile([128, 2], f32)
            xt = xp.tile([128, SPC, 2, H * W], bf16)

            def emit_x_load(s, ci=None):
                for c in ((0, 1) if ci is None else (ci,)):
                    nc.scalar.dma_start(out=xt[:, s, c, :], in_=x_d[s, c])

            def emit_const_loads():
                nc.scalar.dma_start(out=rb_pm[:], in_=rb_d[:])
                nc.scalar.dma_start(out=cb_sb[:], in_=cb_d[:])

            # b2[c, pos] = sum_s bias^2  (sample independent)
            b2 = consts.tile([128, NSLOT, 32], f32)

            def emit_b2():
                rbsq = bigp.tile([128, NSLOT, 256], f32, tag="big1", bufs=2)
                nc.gpsimd.tensor_mul(rbsq[:], rb_pm[:], rb_pm[:])
                nc.vector.tensor_reduce(
                    out=b2[:], in_=rbsq[:].rearrange("p a (c s) -> p a c s", s=OS0),
                    axis=X, op=ALU.add)

            # pos-major caps per sample, filled by both co passes
            caps_pm = [cpmp.tile([128, NSLOT, 256], f32, tag=f"cpm{s}", name=f"cpm{s}")
                       for s in range(SPC)]
            for s in range(SPC):
                # slot 6 partitions >=32 are never written by the transposes
                # (engine APs with partition offset are limited to 32-partition
                # windows, so clear in 32-wide chunks)
                for pb in range(1, 4):
                    nc.gpsimd.memset(caps_pm[s][32 * pb:32 * (pb + 1), NSLOT - 1, :], 0.0)

            def emit_conv_pass(samples, cop):
                """Conv over `samples` for cout half `cop`. Returns psum tiles."""
                ps = [[psp.tile([128, 392], f32, tag="ps", name=f"ps{samples[0]}_{cop}_{s}_{h}")
                       for h in range(2)] for s in range(len(samples))]
                for g0 in range(0, NTAP, TAPG):
                    gn = min(TAPG, NTAP - g0)
                    wg = wp.tile([128, TAPG, 2, 128], bf16, tag="wg",
                                 name=f"wg{samples[0]}_{cop}_{g0}")
                    nc.sync.dma_start(
                        out=wg[:, :gn, :, :],
                        in_=w_d[cop, g0:g0 + gn].transpose([2, 0, 1, 3]))
                    for tl in range(gn):
                        t = g0 + tl
                        kh, kw = t // KK, t % KK
                        for ci in range(2):
                            lhsT = wg[:, tl, ci, :]
                            first = (t == 0 and ci == 0)
                            last = (t == NTAP - 1 and ci == 1)
                            for si, smp in enumerate(samples):
                                xs = xt[:, smp, ci, :]
                                for h in range(2):
                                    rhs = bass.AP(
                                        xs.tensor,
                                        xs.offset + (28 * h + kh) * W + kw,
                                        [xs.ap[0], [2 * W, 14], [2, 28]])
                                    nc.tensor.matmul(
                                        ps[si][h][:], lhsT, rhs,
                                        start=first, stop=last)
                return ps

            def emit_evac_transpose(s, cop, ps_s):
                """PSUM -> caps_cm (conv bias add, on ACT) -> 32x32-block
                transpose into caps_pm[s] columns of this cout half (DVE)."""
                ccm = ccmp.tile([128, POSP], f32, tag="ccm", name=f"ccm{s}_{cop}")
                for h in range(2):
                    nc.scalar.activation(
                        out=ccm[:, 392 * h:392 * (h + 1)], in_=ps_s[h][:],
                        func=AF.Identity, bias=cb_sb[:, cop:cop + 1])
                nc.gpsimd.memset(ccm[:, POS:POSP], 0.0)
                cc = ccm[:]
                cp = caps_pm[s][:]
                pstep_c = cc.ap[0][0]
                pstep_p = cp.ap[0][0]
                for r in range(4):          # co 32-blocks within this half
                    for g in range(4):      # pos partition groups
                        nblk = 7 if g == 0 else 6
                        in_ap = bass.AP(
                            cc.tensor,
                            cc.offset + 32 * r * pstep_c + 32 * g,
                            [[pstep_c, 32], [128, nblk], [1, 32]])
                        out_ap = bass.AP(
                            cp.tensor,
                            cp.offset + 32 * g * pstep_p + cop * 128 + 32 * r,
                            [[pstep_p, 32], [256, nblk], [1, 32]])
                        nc.vector.transpose(out=out_ap, in_=in_ap)

            def bcast_cs(ap):  # [128, NSLOT, 32] -> broadcast over the os dim
                a = ap
                return bass.AP(a.tensor, a.offset,
                               [a.ap[0], a.ap[1], a.ap[2], [0, OS0]])

            def bcast_c(ap):  # [128, NSLOT] -> broadcast over 32 caps types
                a = ap
                return bass.AP(a.tensor, a.offset, [a.ap[0], a.ap[1], [0, 32]])

            # per-(sample, co-half) routing prep: caps-type c < 16 lives wholly
            # in co half 0, c >= 16 in half 1, so caps2/cbr/g0/v2/vb for each
            # half can be computed as soon as that half is transposed (and
            # overlap the other half's conv).
            prep_tiles = {}

            def lnexp_rsqrt(dst, n2ap, bias_ap, scale):
                """dst = exp(scale * ln(n2 + bias)) on ACT only (no table swap
                with the softmax Exp; Sqrt lives in a different table set)."""
                h = scp.tile([128, NSLOT, 32], f32, tag="tp", bufs=12, name="lnh")
                hv = h[:, :, :n2ap.shape[-1]]
                nc.scalar.activation(out=hv, in_=n2ap, func=AF.Ln, bias=bias_ap)
                nc.scalar.activation(out=dst, in_=hv, func=AF.Exp, scale=float(scale))

            def squash_g(gt_ap, n2ap):
                """gt = n2/((1+n2)*sqrt(n2+eps)); rsqrt via exp(-0.5*ln(.))
                (stays in the Exp/Ln table set), 1/(1+n2) via DVE reciprocal
                which overlaps the ACT ops."""
                n = n2ap.shape[-1]
                w = scp.tile([128, NSLOT, 32], f32, tag="tp", bufs=12, name="sqw")
                wv = w[:, :, :n]
                lnexp_rsqrt(wv, n2ap, eps_sb[:], -0.5)     # 1/sqrt(n2+eps)
                t1 = scp.tile([128, NSLOT, 32], f32, tag="tp", bufs=12, name="sqt")
                t1v = t1[:, :, :n]
                nc.vector.tensor_scalar_add(t1v, n2ap, 1.0)
                rq = scp.tile([128, NSLOT, 32], f32, tag="tp", bufs=12, name="sqr")
                rqv = rq[:, :, :n]
                nc.vector.reciprocal(rqv, t1v)
                a = scp.tile([128, NSLOT, 32], f32, tag="tp", bufs=12, name="sqa")
                av = a[:, :, :n]
                nc.vector.tensor_mul(av, n2ap, wv)
                nc.vector.tensor_mul(gt_ap, av, rqv)

            def emit_routing_prep(s, cop):
                if s not in prep_tiles:
                    prep_tiles[s] = dict(
                        g0=scp.tile([128, NSLOT, 32], f32, tag="g0", bufs=2, name=f"g0_{s}"),
                        v2=scp.tile([128, NSLOT, 32], f32, tag="v2", bufs=2, name=f"v2_{s}"),
                        vb=scp.tile([128, NSLOT, 32], f32, tag="vb", bufs=2, name=f"vb_{s}"),
                    )
                pt = prep_tiles[s]
                lo, hi = 16 * cop, 16 * (cop + 1)
                half = caps_pm[s][:, :, 128 * cop:128 * (cop + 1)]
                half4 = half.rearrange("p a (c s) -> p a c s", s=OS0)
                rbh4 = rb_pm[:, :, 128 * cop:128 * (cop + 1)].rearrange(
                    "p a (c s) -> p a c s", s=OS0)
                sqh = bigp.tile([128, NSLOT, 128], f32, tag="bigh", bufs=2,
                                name=f"sqh{s}_{cop}")
                nc.gpsimd.tensor_mul(sqh[:], half, half)
                caps2 = scp.tile([128, NSLOT, 32], f32, tag="tp", bufs=12, name="c2h")
                c2v = caps2[:, :, :16]
                nc.vector.tensor_reduce(
                    out=c2v, in_=sqh[:].rearrange("p a (c s) -> p a c s", s=OS0),
                    axis=X, op=ALU.add)
                sqh2 = bigp.tile([128, NSLOT, 128], f32, tag="bigh", bufs=2,
                                 name=f"sqh2_{s}_{cop}")
                nc.gpsimd.tensor_tensor(
                    out=sqh2[:].rearrange("p a (c s) -> p a c s", s=OS0),
                    in0=half4, in1=rbh4, op=ALU.mult)
                cbr = scp.tile([128, NSLOT, 32], f32, tag="tp", bufs=12, name="cbh")
                cbv = cbr[:, :, :16]
                nc.vector.tensor_reduce(
                    out=cbv, in_=sqh2[:].rearrange("p a (c s) -> p a c s", s=OS0),
                    axis=X, op=ALU.add)
                g0h = pt["g0"][:, :, lo:hi]
                squash_g(g0h, c2v)
                gg = scp.tile([128, NSLOT, 32], f32, tag="tp", bufs=12, name="ggh")
                ggv = gg[:, :, :16]
                nc.vector.tensor_mul(ggv, g0h, g0h)
                nc.vector.tensor_mul(pt["v2"][:, :, lo:hi], ggv, c2v)
                nc.vector.tensor_mul(pt["vb"][:, :, lo:hi], g0h, cbv)

            def emit_routing(s, tail=False):
                # the tail sample runs its poses expansion on the otherwise-idle
                # DVE instead of GPSIMD (shorter serial chain after conv ends)
                pose_eng = nc.vector if tail else nc.gpsimd
                pt = prep_tiles[s]
                g0t, v2, vb = pt["g0"], pt["v2"], pt["vb"]
                caps4 = caps_pm[s][:].rearrange("p a (c s) -> p a c s", s=OS0)

                # ---- iteration 1: coupling c == 1/32 exactly ----
                c0 = 1.0 / 32.0
                tmp = scp.tile([128, NSLOT, 32], f32, tag="tp", bufs=12, name="tmp")
                stt(tmp[:], vb[:], 2.0 * c0, b2[:])
                s2 = scp.tile([128, NSLOT, 32], f32, tag="s2", bufs=2, name=f"s2_{s}_1")
                stt(s2[:], v2[:], c0 * c0, tmp[:])
                gt = scp.tile([128, NSLOT, 32], f32, tag="g", bufs=2, name=f"g_{s}_1")
                squash_g(gt[:], s2[:])
                inner = scp.tile([128, NSLOT, 32], f32, tag="tp", bufs=12, name="inner")
                stt(inner[:], v2[:], c0, vb[:])
                logits = scp.tile([128, NSLOT, 32], f32, tag="logits", bufs=2, name=f"lg{s}_1")
                nc.vector.tensor_mul(logits[:], gt[:], inner[:])

                # ---- iterations 2, 3 ----
                for it in (2, 3):
                    e = scp.tile([128, NSLOT, 32], f32, tag="tp", bufs=12, name="e")
                    nc.scalar.activation(out=e[:], in_=logits[:], func=AF.Exp)
                    den = scp.tile([128, NSLOT], f32, tag="tp", bufs=12, name="den")
                    nc.vector.tensor_reduce(out=den[:], in_=e[:], axis=X, op=ALU.add)
                    rden = scp.tile([128, NSLOT], f32, tag="tp", bufs=12, name="rden")
                    nc.vector.reciprocal(rden[:], den[:])
                    ct = scp.tile([128, NSLOT, 32], f32, tag="ct", bufs=2, name=f"ct{s}_{it}")
                    nc.vector.tensor_tensor(out=ct[:], in0=e[:], in1=bcast_c(rden[:]),
                                            op=ALU.mult)
                    # s2 = c*(c*v2 + 2vb) + b2 ; dot-inner = c*v2 + vb
                    q1 = scp.tile([128, NSLOT, 32], f32, tag="tp", bufs=12, name="q1")
                    nc.vector.tensor_mul(q1[:], v2[:], ct[:])
                    q2 = scp.tile([128, NSLOT, 32], f32, tag="tp", bufs=12, name="q2")
                    stt(q2[:], vb[:], 2.0, q1[:])
                    q3 = scp.tile([128, NSLOT, 32], f32, tag="tp", bufs=12, name="q3")
                    nc.vector.tensor_mul(q3[:], q2[:], ct[:])
                    s2 = scp.tile([128, NSLOT, 32], f32, tag="s2", bufs=2, name=f"s2_{s}_{it}")
                    nc.vector.tensor_add(s2[:], q3[:], b2[:])
                    gt = scp.tile([128, NSLOT, 32], f32, tag="g", bufs=2, name=f"g_{s}_{it}")
                    squash_g(gt[:], s2[:])
                    if it == 2:
                        inner = scp.tile([128, NSLOT, 32], f32, tag="tp", bufs=12, name="i2")
                        nc.vector.tensor_add(inner[:], q1[:], vb[:])
                        dot = scp.tile([128, NSLOT, 32], f32, tag="tp", bufs=12, name="dot")
                        nc.vector.tensor_mul(dot[:], gt[:], inner[:])
                        logits2 = scp.tile([128, NSLOT, 32], f32, tag="logits", bufs=2,
                                           name=f"lg{s}_2")
                        nc.vector.tensor_add(logits2[:], logits[:], dot[:])
                        logits = logits2

                # ---- outputs ----
                w_ = scp.tile([128, NSLOT, 32], f32, tag="tp", bufs=12, name="w_")
                lnexp_rsqrt(w_[:], s2[:], tiny_sb[:], 0.5)   # sqrt(s2) (pad lanes: s2=0)
                acts_sb = scp.tile([128, NSLOT, 32], f32, tag="acts", bufs=2, name=f"acts{s}")
                nc.vector.tensor_mul(acts_sb[:], gt[:], w_[:])
                a1 = scp.tile([128, NSLOT, 32], f32, tag="tp", bufs=12, name="a1")
                nc.vector.tensor_mul(a1[:], gt[:], ct[:])
                Acoef = scp.tile([128, NSLOT, 32], f32, tag="tp", bufs=12, name="Ac")
                nc.vector.tensor_mul(Acoef[:], a1[:], g0t[:])

                nc.scalar.dma_start(out=acts_d[s], in_=acts_sb[:])
                # poses = A*caps + g*bias, in two slot chunks so the first
                # chunk's DMA overlaps the second chunk's compute
                for sl0, sl1 in ((0, 4), (4, NSLOT)):
                    tmpP = bigp.tile([128, NSLOT, 256], f32, tag="big1", bufs=2,
                                     name=f"tp_{s}_{sl0}")
                    pose_eng.tensor_tensor(
                        out=tmpP[:, sl0:sl1].rearrange("p a (c s) -> p a c s", s=OS0),
                        in0=caps4[:, sl0:sl1], in1=bcast_cs(Acoef[:])[:, sl0:sl1],
                        op=ALU.mult)
                    tmpQ = bigp.tile([128, NSLOT, 256], f32, tag="big2", bufs=1,
                                     name=f"tq{s}_{sl0}")
                    nc.gpsimd.tensor_tensor(
                        out=tmpQ[:, sl0:sl1].rearrange("p a (c s) -> p a c s", s=OS0),
                        in0=rb_pm[:, sl0:sl1].rearrange("p a (c s) -> p a c s", s=OS0),
                        in1=bcast_cs(gt[:])[:, sl0:sl1], op=ALU.mult)
                    pose_eng.tensor_add(tmpP[:, sl0:sl1], tmpP[:, sl0:sl1],
                                        tmpQ[:, sl0:sl1])
                    nc.scalar.dma_start(out=poses_d[s][:, sl0:sl1], in_=tmpP[:, sl0:sl1])

            # =====================  schedule  =====================
            # pair pass for samples 0/1 (best weight-load amortization while
            # nothing else competes), then per-sample passes for 2 and 3 so
            # only the last sample's routing sits past the end of the conv.
            emit_x_load(0)
            emit_x_load(1)
            emit_const_loads()
            emit_b2()
            for cop in range(2):
                ps = emit_conv_pass((0, 1), cop)
                if cop == 0:
                    emit_x_load(2)
                    emit_x_load(3)
                for si in range(2):
                    emit_evac_transpose(si, cop, ps[si])
                    emit_routing_prep(si, cop)
            for si in range(2):
                emit_routing(si)
            for smp in (2, 3):
                for cop in range(2):
                    ps = emit_conv_pass((smp,), cop)
                    emit_evac_transpose(smp, cop, ps[0])
                    emit_routing_prep(smp, cop)
                emit_routing(smp, tail=(smp == 3))

    nc.compile()
    return nc


def get_program():
    global _PROG
    if _PROG is None:
        _PROG = _build_program()
    return _PROG


def build_inmaps(x, conv_w, conv_b, routing_bias):
    x = np.asarray(x, dtype=np.float32)
    conv_w = np.asarray(conv_w, dtype=np.float32)
    conv_b = np.asarray(conv_b, dtype=np.float32)
    routing_bias = np.asarray(routing_bias, dtype=np.float32)

    xb = x.astype(ml_dtypes.bfloat16).reshape(B, 2, 128, H * W)
    # conv_w [cout, cin, kh, kw] -> [cop, tap, cichunk, ci, co]
    wt = np.ascontiguousarray(
        conv_w.reshape(256, 256, NTAP).transpose(2, 1, 0)  # [tap, ci, co]
        .reshape(NTAP, 2, 128, 2, 128)                     # [tap, cich, ci, coch, co]
        .transpose(3, 0, 1, 2, 4)                          # [coch, tap, cich, ci, co]
    ).astype(ml_dtypes.bfloat16)
    # routing_bias [32, 28, 28, 8, 1] -> pos-major [128, slot, 256]
    rb = routing_bias.reshape(C_TYPES, OH, OW, OS0).transpose(1, 2, 0, 3).reshape(POS, 256)
    rbp = np.zeros((NSLOT * 128, 256), np.float32)
    rbp[:POS] = rb
    rb_h = np.ascontiguousarray(rbp.reshape(NSLOT, 128, 256).transpose(1, 0, 2))
    cb_h = np.ascontiguousarray(conv_b.reshape(2, 128).T)

    in_maps = []
    for c in range(NCORES):
        in_maps.append({
            "xb": np.ascontiguousarray(xb[SPC * c:SPC * (c + 1)]),
            "wt": wt,
            "rb": rb_h,
            "cb": cb_h,
        })
    return in_maps


def postprocess(results):
    poses_all = []
    acts_all = []
    for r in results:
        pr = np.asarray(r["poses"])   # [SPC, 128, NSLOT, 256]
        ar = np.asarray(r["acts"])    # [SPC, 128, NSLOT, 32]
        p = pr.transpose(0, 2, 1, 3).reshape(SPC, NSLOT * 128, 256)[:, :POS]
        p = p.reshape(SPC, OH, OW, C_TYPES, OS0).transpose(0, 3, 1, 2, 4)[..., None]
        a = ar.transpose(0, 2, 1, 3).reshape(SPC, NSLOT * 128, 32)[:, :POS]
        a = a.reshape(SPC, OH, OW, C_TYPES).transpose(0, 3, 1, 2)
        poses_all.append(p)
        acts_all.append(a)
    poses = np.concatenate(poses_all, axis=0).astype(np.float32)
    acts = np.concatenate(acts_all, axis=0).astype(np.float32)
    return poses, acts


def kernel(x, conv_w, conv_b, routing_bias):
    from concourse.bass_utils import run_bass_kernel_spmd

    nc = get_program()
    in_maps = build_inmaps(x, conv_w, conv_b, routing_bias)
    trace = bool(int(os.environ.get("CAPS_TRACE", "0")))
    try:
        res = run_bass_kernel_spmd(nc, in_maps, list(range(NCORES)), trace=trace)
    except ModuleNotFoundError:
        # axon NTFF profiling hook not shipped in this container
        res = run_bass_kernel_spmd(nc, in_maps, list(range(NCORES)), trace=False)
    kernel.last_results = res
    return postprocess(res.results)


def cost_model_time_ns():
    """Per-core kernel time from the CoreSim cost model (the container's axon
    client cannot fetch NTFF profiles, so this is the best timing estimate;
    inputs are zeros — timing does not depend on data values)."""
    from concourse.bass_interp import CoreSim

    nc = get_program()
    sim = CoreSim(nc, trace=False, publish_trace=False)
    zeros = build_inmaps(
        np.zeros((B, CIN, H, W), np.float32),
        np.zeros((256, 256, KK, KK), np.float32),
        np.zeros((256,), np.float32),
        np.zeros((C_TYPES, OH, OW, OS0, 1), np.float32))[0]
    sim.assign_tensors(zeros)
    sim.simulate()
    return int(sim.time)
